# revision 1
# baseline (speedup 1.0000x reference)
"""Trainium2 Bass kernel for nn_CrossAttentionLayer (dual-stream transformer block).

Sharding (8 NeuronCores, one SPMD program, identical per core):
  * Both streams sharded by the L=1024 position dim: core i owns t in
    [i*128,(i+1)*128) for all 4 batch elements -> 512 "rows"/stream/core.
  * Self-attention (seq-first nn.MultiheadAttention: attends over dim0 of size
    B=4, batched over positions) is local under the t-shard; computed with DVE
    elementwise ops over the 16 (s,u) batch pairs.
  * Cross-attention needs full-length K/V per batch: K^T and V(+ones column)
    are projected on local rows and AllGathered in fp16.
    Scores are computed transposed (S^T: k on partitions, q on free) so softmax
    needs no transposes/partition reductions: exp on ACT (max-subtract skipped;
    |scores| < ~4), denominator from the ones column in the AV matmul,
    normalization via a 1-row PE broadcast matmul.
  * Matmuls in fp16 (fp32 PSUM accumulate); LN/softmax stats + residuals fp32.
  * Weights stream through one 24KB/partition double-buffered SBUF slot;
    inter-phase activations round-trip through DRAM (fp16).
"""

import numpy as np
from contextlib import ExitStack

import concourse.bass as bass
import concourse.bacc as bacc
import concourse.mybir as mybir
import concourse.tile as tile
from concourse import bass_utils

H = 16
D = 1024
B = 4
HD = 64
EPS = 1e-5
P = 128
KT = D // P  # 8 contraction tiles over D

F16 = mybir.dt.float16
F32 = mybir.dt.float32
AF = mybir.ActivationFunctionType
ALU = mybir.AluOpType
X_AX = mybir.AxisListType.X


def _bcast_part(ap, n):
    return bass.AP(tensor=ap.tensor, offset=ap.offset, ap=[[0, n]] + list(ap.ap))


def _bcast_last(ap, n):
    return bass.AP(tensor=ap.tensor, offset=ap.offset, ap=list(ap.ap) + [[0, n]])


class Emitter:
    def __init__(self, tc, ctx, io, NC, triv):
        self.tc = tc
        self.nc = tc.nc
        self.io = io
        self.NC = NC
        self.R = B * P
        self.triv = triv

        nc = self.nc
        self.const = ctx.enter_context(tc.tile_pool(name="const", bufs=1))
        self.act = ctx.enter_context(tc.tile_pool(name="act", bufs=1))
        self.act2 = ctx.enter_context(tc.tile_pool(name="act2", bufs=2))
        self.act3 = ctx.enter_context(tc.tile_pool(name="act3", bufs=3))
        self.w2 = ctx.enter_context(tc.tile_pool(name="w2", bufs=2))
        self.dram = ctx.enter_context(tc.tile_pool(name="dram", bufs=1, space="DRAM"))
        self.pp = ctx.enter_context(tc.tile_pool(name="pp", bufs=2, space="PSUM"))
        self.pp_s = ctx.enter_context(tc.tile_pool(name="pp_s", bufs=2, space="PSUM"))
        self.pp_o = ctx.enter_context(tc.tile_pool(name="pp_o", bufs=1, space="PSUM"))
        self.pp_sm = ctx.enter_context(tc.tile_pool(name="pp_sm", bufs=1, space="PSUM"))

        self.ident = self.const.tile([P, P], F16)
        nc.sync.dma_start(out=self.ident[:], in_=io["ident16"])
        self.epst = self.const.tile([P, 1], F32)
        nc.vector.memset(self.epst[:], EPS)
        self.ones_col = self.const.tile([P, HD], F16)
        nc.vector.memset(self.ones_col[:], 1.0)

        def rep(name, n):
            t = self.const.tile([P, n], F16)
            nc.gpsimd.dma_start(out=t[:], in_=_bcast_part(io[name], P))
            return t

        self.sa_in_b_rep = None if triv["sa_in_b"] else rep("sa_in_b16", 3 * D)
        self.cv_b_rep = None if triv["cv_b"] else rep("cv_b16", D)
        self.co_b_rep = None if triv["co_b"] else rep("co_b16", D)
        self.b_rep = {}
        for bn in ["b1", "b2", "b3", "b4"]:
            self.b_rep[bn] = None if triv[bn] else rep(bn + "_16", D)
        self.ln_reps = {}
        for lnm in ["n1v", "n2v", "n3v", "n1t", "n2t", "n3t"]:
            if not triv[lnm]:
                self.ln_reps[lnm] = (rep(lnm + "_g16", D), rep(lnm + "_b16", D))
        self.cqbT = self.ckbT = None
        if not triv["cq_b"]:
            self.cqbT = self.const.tile([P, KT], F32)
            nc.sync.dma_start(out=self.cqbT[:], in_=io["cq_bT"])
        if not triv["ck_b"]:
            self.ckbT = self.const.tile([P, KT], F32)
            nc.sync.dma_start(out=self.ckbT[:], in_=io["ck_bT"])

    # ---- weight streaming through the shared 16KB "w" slot (bufs=2) ----
    def load_w(self, name, n_out, third=None):
        """DRAM (D, n_out) fp16 -> SBUF (128, KT, n_out) [or a 1024-wide third]."""
        t = self.w2.tile([P, KT, D], F16, tag="w")
        src = self.io[name].rearrange("(k p) n -> p k n", p=P)
        if third is None:
            self.nc.sync.dma_start(out=t[:], in_=src)
        else:
            self.nc.sync.dma_start(out=t[:], in_=src[:, :, third * D:(third + 1) * D])
        return t

    def load_co_half(self, half):
        """coT64_16 DRAM (64, 16, D) -> SBUF (64, 8, D), heads half*8..+8."""
        t = self.w2.tile([HD, 8, D], F16, tag="w")
        self.nc.sync.dma_start(
            out=t[:], in_=self.io["coT64_16"][:, half * 8:(half + 1) * 8, :])
        return t

    def load_xTs(self, xT_dram):
        """x^T DRAM (D, R) f16 -> SBUF (128, KT, R)."""
        t = self.act.tile([P, KT, self.R], F16, tag="xTs")
        self.nc.sync.dma_start(
            out=t[:], in_=xT_dram[:].rearrange("(k p) r -> p k r", p=P)
            if hasattr(xT_dram, "shape") else xT_dram)
        return t

    def evict_add(self, out_ap, psum_ap, rep_slice):
        if rep_slice is None:
            self.nc.vector.tensor_copy(out_ap, psum_ap)
        else:
            self.nc.vector.tensor_add(out_ap, psum_ap, rep_slice)

    # ---- layernorm over free dim of an f32 (128, 1024) tile ----
    def emit_ln(self, r, out, lnm, relu=False):
        nc = self.nc
        st = self.act.tile([P, 2, 6], F32, tag="ln_st")
        rv = r[:].rearrange("p (c n) -> p c n", c=2)
        nc.vector.bn_stats(st[:, 0, :], rv[:, 0, :])
        nc.vector.bn_stats(st[:, 1, :], rv[:, 1, :])
        mv = self.act.tile([P, 2], F32, tag="ln_mv")
        nc.vector.bn_aggr(mv[:], st[:])
        sd = self.act.tile([P, 1], F32, tag="ln_sd")
        nc.scalar.activation(sd[:], mv[:, 1:2], AF.Sqrt, bias=self.epst[:])
        rs = self.act.tile([P, 1], F32, tag="ln_rs")
        nc.vector.reciprocal(rs[:], sd[:])
        nm = self.act.tile([P, 1], F32, tag="ln_nm")
        nc.vector.scalar_tensor_tensor(
            out=nm[:], in0=mv[:, 0:1], scalar=-1.0, in1=rs[:],
            op0=ALU.mult, op1=ALU.mult)
        if self.triv[lnm]:
            nc.scalar.activation(out[:], r[:], AF.Relu if relu else AF.Identity,
                                 bias=nm[:], scale=rs[:])
            return
        y = self.act.tile([P, D], F32, tag="ln_y")
        nc.scalar.activation(y[:], r[:], AF.Identity, bias=nm[:], scale=rs[:])
        g_rep, b_rep = self.ln_reps[lnm]
        nc.vector.tensor_mul(y[:], y[:], g_rep[:])
        nc.vector.tensor_add(y[:], y[:], b_rep[:])
        if relu:
            nc.vector.tensor_scalar_max(out[:], y[:], 0.0)
        else:
            nc.vector.tensor_copy(out[:], y[:])

    # ---- transpose one (128,1024) f16 rows tile into DRAM xT (D, R) ----
    def transpose_rows_to_dram(self, rows16, b, xT_dram):
        nc = self.nc
        for dt in range(KT):
            tp = self.pp_sm.tile([P, P], F16, tag="sm_ps")
            nc.tensor.transpose(tp[:], rows16[:, dt * P:(dt + 1) * P], self.ident[:])
            tev = self.act3.tile([P, P], F16, tag="tev")
            nc.vector.tensor_copy(tev[:], tp[:])
            nc.sync.dma_start(
                out=xT_dram[dt * P:(dt + 1) * P, b * P:(b + 1) * P], in_=tev[:])

    # ---- self-attention + LN1 for one stream ----
    def emit_sa_stream(self, s, sa_pool, lnm):
        nc = self.nc
        io = self.io

        qkv = [sa_pool.tile([P, 3 * D], F16, tag=f"qkv{b}", name=f"qkv{b}") for b in range(B)]
        xbs = []
        for b in range(B):
            xb = sa_pool.tile([P, KT, P], F16, tag=f"xTb{b}")
            nc.sync.dma_start(
                out=xb[:],
                in_=io[f"xT16_{s}"][:, b * P:(b + 1) * P].rearrange(
                    "(k p) r -> p k r", p=P))
            xbs.append(xb)
        for third in range(3):
            sw = self.load_w("sawT16", None, third=third)
            for b in range(B):
                for n2 in range(2):
                    ps = self.pp.tile([P, 512], F32, tag="proj_ps")
                    for k in range(KT):
                        nc.tensor.matmul(
                            ps[:], lhsT=xbs[b][:, k, :],
                            rhs=sw[:, k, n2 * 512:(n2 + 1) * 512],
                            start=(k == 0), stop=(k == KT - 1))
                    off = third * D + n2 * 512
                    self.evict_add(
                        qkv[b][:, off:off + 512], ps[:],
                        None if self.sa_in_b_rep is None
                        else self.sa_in_b_rep[:, off:off + 512])

        sc = sa_pool.tile([P, B, H, B], F32, tag="sa_sc")
        for sq in range(B):
            for u in range(B):
                pt = sa_pool.tile([P, D], F16, tag="sa_pt")
                nc.vector.tensor_mul(pt[:], qkv[sq][:, 0:D], qkv[u][:, D:2 * D])
                nc.vector.reduce_sum(
                    out=sc[:, sq, :, u],
                    in_=pt[:].rearrange("p (h d) -> p h d", h=H), axis=X_AX)
        esc = sa_pool.tile([P, B, H, B], F32, tag="sa_esc")
        nc.scalar.activation(esc[:], sc[:], AF.Exp, scale=0.125)
        den = sa_pool.tile([P, B, H], F32, tag="sa_den")
        nc.vector.reduce_sum(out=den[:], in_=esc[:], axis=X_AX)
        rden = sa_pool.tile([P, B, H], F32, tag="sa_rden")
        nc.vector.reciprocal(rden[:], den[:])
        a16 = sa_pool.tile([P, B, H, B], F16, tag="sa_a16")
        nc.vector.tensor_mul(a16[:], esc[:], _bcast_last(rden[:], B))

        o16 = []
        for sq in range(B):
            o = sa_pool.tile([P, D], F16, tag=f"sa_o{sq}")
            o16.append(o)
            ov = o[:].rearrange("p (h d) -> p h d", h=H)
            tmp = sa_pool.tile([P, D], F16, tag="sa_tmp")
            tv = tmp[:].rearrange("p (h d) -> p h d", h=H)
            for u in range(B):
                vv = qkv[u][:, 2 * D:3 * D].rearrange("p (h d) -> p h d", h=H)
                av = _bcast_last(a16[:, sq, :, u], HD)
                if u == 0:
                    nc.vector.tensor_mul(ov, vv, av)
                else:
                    nc.vector.tensor_mul(tv, vv, av)
                    nc.vector.tensor_add(ov, ov, tv)

        # transpose attention output
        oT = sa_pool.tile([P, KT, B, P], F16, tag="oT")
        for b in range(B):
            for dt in range(KT):
                tp = self.pp_sm.tile([P, P], F16, tag="sm_ps")
                nc.tensor.transpose(tp[:], o16[b][:, dt * P:(dt + 1) * P], self.ident[:])
                nc.vector.tensor_copy(oT[:, dt, b, :], tp[:])

        # out-proj + residual + LN1 -> x1b_dram (f32 rows) and x1T_dram (f16)
        so = self.load_w("saoT16", D)
        x1b_dram = self.dram.tile([self.R, D], F32, tag=f"x1b_{s}")
        x1T_dram = self.dram.tile([D, self.R], F16, tag=f"x1T_{s}")
        for b in range(B):
            rowsb = sa_pool.tile([P, D], F32, tag="rowsb")
            nc.sync.dma_start(out=rowsb[:], in_=io[f"rowsb_{s}"][b * P:(b + 1) * P, :])
            r = sa_pool.tile([P, D], F32, tag="r1")
            for nch in range(2):
                ps = self.pp.tile([P, 512], F32, tag="proj_ps")
                for dt in range(KT):
                    nc.tensor.matmul(
                        ps[:], lhsT=oT[:, dt, b, :],
                        rhs=so[:, dt, nch * 512:(nch + 1) * 512],
                        start=(dt == 0), stop=(dt == KT - 1))
                sl = slice(nch * 512, (nch + 1) * 512)
                nc.vector.tensor_add(r[:, sl], ps[:], rowsb[:, sl])
            x1 = sa_pool.tile([P, D], F32, tag="x1")
            self.emit_ln(r, x1, lnm)
            if self.co_b_rep is not None:
                xb1 = sa_pool.tile([P, D], F32, tag="x1b_t")
                nc.vector.tensor_add(xb1[:], x1[:], self.co_b_rep[:])
                nc.sync.dma_start(out=x1b_dram[b * P:(b + 1) * P, :], in_=xb1[:])
            else:
                nc.sync.dma_start(out=x1b_dram[b * P:(b + 1) * P, :], in_=x1[:])
            x16 = self.act2.tile([P, D], F16, tag="x_16")
            nc.vector.tensor_copy(x16[:], x1[:])
            self.transpose_rows_to_dram(x16, b, x1T_dram)
        return x1b_dram, x1T_dram

    # ---- projections from a loaded transposed activation (xTs SBUF) ----
    def proj_to_dram(self, xTs, wT, biasT, out_dram):
        """out_dram (D, R) f16 = wT.T @ x."""
        nc = self.nc
        for ot in range(KT):
            ps = self.pp.tile([P, self.R], F32, tag="proj_ps")
            for k in range(KT):
                nc.tensor.matmul(
                    ps[:], lhsT=wT[:, k, ot * P:(ot + 1) * P], rhs=xTs[:, k, :],
                    start=(k == 0), stop=(k == KT - 1))
            t = self.act3.tile([P, self.R], F16, tag="pev")
            if biasT is not None:
                nc.scalar.activation(t[:], ps[:], AF.Identity, bias=biasT[:, ot:ot + 1])
            else:
                nc.scalar.copy(t[:], ps[:])
            nc.sync.dma_start(out=out_dram[ot * P:(ot + 1) * P, :], in_=t[:])

    def emit_kv_and_ag(self, xTs, which):
        """K^T/V_ext from loaded xTs -> DRAM -> AllGather."""
        nc = self.nc
        NC = self.NC
        ck = self.load_w("ckT16", D)
        kT_loc = self.dram.tile([D, self.R], F16, tag=f"kloc{which}")
        self.proj_to_dram(xTs, ck, self.ckbT, kT_loc)
        cv = self.load_w("cvT16", D)
        v_loc = self.dram.tile([self.R, H * 65], F16, tag=f"vloc{which}")
        for b in range(B):
            ve = self.act2.tile([P, H, 65], F16, tag="ve")
            for nch in range(2):
                ps = self.pp.tile([P, 512], F32, tag="proj_ps")
                for k in range(KT):
                    nc.tensor.matmul(
                        ps[:], lhsT=xTs[:, k, b * P:(b + 1) * P],
                        rhs=cv[:, k, nch * 512:(nch + 1) * 512],
                        start=(k == 0), stop=(k == KT - 1))
                dst = ve[:, nch * 8:(nch + 1) * 8, 0:HD]
                src = ps[:].rearrange("p (h d) -> p h d", h=8)
                if self.cv_b_rep is None:
                    nc.vector.tensor_copy(dst, src)
                else:
                    nc.vector.tensor_add(
                        dst, src,
                        self.cv_b_rep[:, nch * 512:(nch + 1) * 512].rearrange(
                            "p (h d) -> p h d", h=8))
            nc.vector.memset(ve[:, :, 64:65], 1.0)
            nc.sync.dma_start(out=v_loc[b * P:(b + 1) * P, :], in_=ve[:])
        kT_g = self.dram.tile([NC * D, self.R], F16, tag=f"kg{which}", addr_space="Shared")
        v_g = self.dram.tile([NC * self.R, H * 65], F16, tag=f"vg{which}", addr_space="Shared")
        rg = [list(range(NC))]
        nc.gpsimd.collective_compute(
            "AllGather", ALU.bypass, replica_groups=rg,
            ins=[kT_loc[:].opt()], outs=[kT_g[:].opt()])
        nc.gpsimd.collective_compute(
            "AllGather", ALU.bypass, replica_groups=rg,
            ins=[v_loc[:].opt()], outs=[v_g[:].opt()])
        return kT_g, v_g

    # ---- cross-attention + LN2 ----
    def emit_ca(self, kv_pool, ca2, q_dram, kT_g, v_g, x1b_dram, lnm, x2_dram, x2T_dram):
        nc = self.nc
        NC = self.NC
        OnT = self.act.tile([HD, H, B, P], F16, tag="OnT")
        for b in range(B):
            qblk = ca2.tile([P, KT, P], F16, tag="qblk")
            nc.sync.dma_start(
                out=qblk[:],
                in_=q_dram[:, b * P:(b + 1) * P].rearrange("(ot p) j -> p ot j", p=P))
            kblk, vblk = [], []
            for c in range(NC):
                kb = kv_pool.tile([P, KT, P], F16, tag="kblk")
                nc.sync.dma_start(
                    out=kb[:],
                    in_=kT_g[c * D:(c + 1) * D, b * P:(b + 1) * P].rearrange(
                        "(dt p) j -> p dt j", p=P))
                kblk.append(kb)
                vb = kv_pool.tile([P, H, 65], F16, tag="vblk")
                nc.sync.dma_start(
                    out=vb[:],
                    in_=v_g[c * self.R + b * P: c * self.R + (b + 1) * P, :].rearrange(
                        "p (h e) -> p h e", h=H))
                vblk.append(vb)
            for h in range(H):
                po = (h % 2) * HD
                ps_s = self.pp_s.tile([P, NC, P], F32, tag="s_ps")
                for c in range(NC):
                    nc.tensor.matmul(
                        ps_s[:, c, :],
                        lhsT=kblk[c][po:po + HD, h // 2, :],
                        rhs=qblk[po:po + HD, h // 2, :],
                        start=True, stop=True)
                aT = ca2.tile([P, NC, P], F16, tag="aT")
                nc.scalar.activation(aT[:], ps_s[:], AF.Exp, scale=0.125)
                ps_o = self.pp_o.tile([65, P], F32, tag="o_ps")
                for c in range(NC):
                    nc.tensor.matmul(
                        ps_o[:], lhsT=vblk[c][:, h, :], rhs=aT[:, c, :],
                        start=(c == 0), stop=(c == NC - 1))
                o_raw = ca2.tile([65, P], F32, tag="o_raw")
                nc.vector.tensor_copy(o_raw[:], ps_o[:])
                nc.vector.reciprocal(o_raw[64:65, :], o_raw[64:65, :])
                rcp = ca2.tile([65, P], F16, tag="rcp")
                nc.vector.tensor_copy(rcp[64:65, :], o_raw[64:65, :])
                bc = self.pp_sm.tile([HD, P], F32, tag="sm_ps")
                nc.tensor.matmul(
                    bc[:], lhsT=self.ones_col[64:65, 0:HD], rhs=rcp[64:65, :],
                    start=True, stop=True)
                nc.vector.tensor_mul(OnT[:, h, b, :], o_raw[0:HD, :], bc[:])
        # out-projection (contract 64 per head, two 8-head weight pieces)
        coA = self.load_co_half(0)
        coB = self.load_co_half(1)
        for b in range(B):
            res = self.act2.tile([P, D], F32, tag="res_in")
            nc.sync.dma_start(out=res[:], in_=x1b_dram[b * P:(b + 1) * P, :])
            r = self.act.tile([P, D], F32, tag="r2")
            for nch in range(2):
                ps = self.pp.tile([P, 512], F32, tag="proj_ps")
                for h in range(H):
                    co = coA if h < 8 else coB
                    nc.tensor.matmul(
                        ps[:], lhsT=OnT[:, h, b, :],
                        rhs=co[:, h % 8, nch * 512:(nch + 1) * 512],
                        start=(h == 0), stop=(h == H - 1))
                sl = slice(nch * 512, (nch + 1) * 512)
                nc.vector.tensor_add(r[:, sl], ps[:], res[:, sl])
            x2 = self.act.tile([P, D], F32, tag="x2")
            self.emit_ln(r, x2, lnm)
            nc.sync.dma_start(out=x2_dram[b * P:(b + 1) * P, :], in_=x2[:])
            x16 = self.act2.tile([P, D], F16, tag="x_16")
            nc.vector.tensor_copy(x16[:], x2[:])
            self.transpose_rows_to_dram(x16, b, x2T_dram)

    # ---- FFN ----
    def emit_ffn(self, xTs, x_dram, w1n, w2n, b1n, b2n, lnm, out_dram):
        nc = self.nc
        w1 = self.load_w(w1n, D)
        w2 = self.load_w(w2n, D)
        b1_rep = self.b_rep[b1n]
        b2_rep = self.b_rep[b2n]
        for b in range(B):
            h1 = self.act.tile([P, D], F32, tag="ffn_h1")
            for nch in range(2):
                ps = self.pp.tile([P, 512], F32, tag="proj_ps")
                for k in range(KT):
                    nc.tensor.matmul(
                        ps[:], lhsT=xTs[:, k, b * P:(b + 1) * P],
                        rhs=w1[:, k, nch * 512:(nch + 1) * 512],
                        start=(k == 0), stop=(k == KT - 1))
                sl = slice(nch * 512, (nch + 1) * 512)
                self.evict_add(h1[:, sl], ps[:],
                               b1_rep[:, sl] if b1_rep is not None else None)
            hr = self.act.tile([P, D], F16, tag="ffn_hr")
            self.emit_ln(h1, hr, lnm, relu=True)
            hT = self.act.tile([P, KT, P], F16, tag="ffn_hT")
            for dt in range(KT):
                tp = self.pp_sm.tile([P, P], F16, tag="sm_ps")
                nc.tensor.transpose(tp[:], hr[:, dt * P:(dt + 1) * P], self.ident[:])
                nc.vector.tensor_copy(hT[:, dt, :], tp[:])
            res = self.act2.tile([P, D], F32, tag="res_in")
            nc.sync.dma_start(out=res[:], in_=x_dram[b * P:(b + 1) * P, :])
            oo = self.act.tile([P, D], F32, tag="ffn_oo")
            for nch in range(2):
                ps = self.pp.tile([P, 512], F32, tag="proj_ps")
                for dt in range(KT):
                    nc.tensor.matmul(
                        ps[:], lhsT=hT[:, dt, :],
                        rhs=w2[:, dt, nch * 512:(nch + 1) * 512],
                        start=(dt == 0), stop=(dt == KT - 1))
                sl = slice(nch * 512, (nch + 1) * 512)
                nc.vector.tensor_add(oo[:, sl], ps[:], res[:, sl])
                if b2_rep is not None:
                    nc.vector.tensor_add(oo[:, sl], oo[:, sl], b2_rep[:, sl])
            nc.sync.dma_start(out=out_dram[b, :, :], in_=oo[:])


def emit_full(tc, io, NC, triv):
    with ExitStack() as ctx:
        em = Emitter(tc, ctx, io, NC, triv)
        R = em.R

        with tc.tile_pool(name="sa_pool", bufs=1) as sa_pool:
            text1b, text1T = em.emit_sa_stream("text", sa_pool, "n1t")
            img1b, img1T = em.emit_sa_stream("img", sa_pool, "n1v")

        with tc.tile_pool(name="kv_pool", bufs=NC + 1) as kv_pool, \
             tc.tile_pool(name="ca2", bufs=2) as ca2:
            xts_t = em.load_xTs(text1T)
            k1g, v1g = em.emit_kv_and_ag(xts_t, 1)
            cq = em.load_w("cqT16", D)
            q2_dram = em.dram.tile([D, R], F16, tag="q2")
            em.proj_to_dram(xts_t, cq, em.cqbT, q2_dram)
            xts_i = em.load_xTs(img1T)
            q1_dram = em.dram.tile([D, R], F16, tag="q1")
            em.proj_to_dram(xts_i, cq, em.cqbT, q1_dram)

            img2_dram = em.dram.tile([R, D], F32, tag="img2")
            img2T_dram = em.dram.tile([D, R], F16, tag="img2T")
            em.emit_ca(kv_pool, ca2, q1_dram, k1g, v1g, img1b, "n2v",
                       img2_dram, img2T_dram)

            xts_i2 = em.load_xTs(img2T_dram)
            k2g, v2g = em.emit_kv_and_ag(xts_i2, 2)
            em.emit_ffn(xts_i2, img2_dram, "w1T16", "w2T16", "b1", "b2", "n3v",
                        io["img_out"])

            text2_dram = em.dram.tile([R, D], F32, tag="text2")
            text2T_dram = em.dram.tile([D, R], F16, tag="text2T")
            em.emit_ca(kv_pool, ca2, q2_dram, k2g, v2g, text1b, "n2t",
                       text2_dram, text2T_dram)

            xts_t2 = em.load_xTs(text2T_dram)
            em.emit_ffn(xts_t2, text2_dram, "w3T16", "w4T16", "b3", "b4", "n3t",
                        io["text_out"])


# ======================= host side =======================

def _triviality(inputs):
    t = {}
    for lnm in ["n1v", "n2v", "n3v", "n1t", "n2t", "n3t"]:
        t[lnm] = bool(np.all(inputs[lnm + "_g"] == 1.0) and
                      np.all(inputs[lnm + "_b"] == 0.0))
    for bn in ["sa_in_b", "cv_b", "co_b", "cq_b", "ck_b", "b1", "b2", "b3", "b4"]:
        t[bn] = bool(np.all(inputs[bn] == 0.0))
    return t


def _host_prep(inputs, NC):
    f16, f32 = np.float16, np.float32
    c = {}
    c["ident16"] = np.eye(P, dtype=f16)
    c["sawT16"] = np.ascontiguousarray(inputs["sa_in_w"].T.astype(f16))
    c["saoT16"] = np.ascontiguousarray(inputs["sa_out_w"].T.astype(f16))
    c["cqT16"] = np.ascontiguousarray(inputs["cq_w"].T.astype(f16))
    c["ckT16"] = np.ascontiguousarray(inputs["ck_w"].T.astype(f16))
    c["cvT16"] = np.ascontiguousarray(inputs["cv_w"].T.astype(f16))
    coT = inputs["co_w"].T.astype(f16)  # (hd, od)
    c["coT64_16"] = np.ascontiguousarray(coT.reshape(H, HD, D).transpose(1, 0, 2))
    for w in ["w1", "w2", "w3", "w4"]:
        c[w + "T16"] = np.ascontiguousarray(inputs[w].T.astype(f16))
    c["sa_in_b16"] = inputs["sa_in_b"].astype(f16)
    for bn in ["cv_b", "co_b"]:
        c[bn + "16"] = inputs[bn].astype(f16)
    for bn in ["b1", "b2", "b3", "b4"]:
        c[bn + "_16"] = inputs[bn].astype(f16)
    c["cq_bT"] = np.ascontiguousarray(inputs["cq_b"].astype(f32).reshape(KT, P).T)
    c["ck_bT"] = np.ascontiguousarray(inputs["ck_b"].astype(f32).reshape(KT, P).T)
    for lnm in ["n1v", "n2v", "n3v", "n1t", "n2t", "n3t"]:
        c[lnm + "_g16"] = inputs[lnm + "_g"].astype(f16)
        c[lnm + "_b16"] = inputs[lnm + "_b"].astype(f16)

    sa_out_b = inputs["sa_out_b"].astype(f32)
    per_core = []
    L = inputs["img_input"].shape[1]
    TC = L // NC
    for ci in range(NC):
        m = {}
        for s, key in [("img", "img_input"), ("text", "text_input")]:
            x = np.asarray(inputs[key][:, ci * TC:(ci + 1) * TC, :], dtype=f32)
            rows = x.reshape(B * TC, D)
            m[f"xT16_{s}"] = np.ascontiguousarray(rows.T.astype(f16))
            m[f"rowsb_{s}"] = np.ascontiguousarray(rows + sa_out_b)
        per_core.append(m)
    return c, per_core


def make_nc(NC, triv):
    R = B * P
    nc = bacc.Bacc("TRN2", target_bir_lowering=False, debug=False, num_devices=NC)
    io = {}

    def din(name, shape, dt):
        io[name] = nc.dram_tensor(name, list(shape), dt, kind="ExternalInput").ap()

    din("ident16", (P, P), F16)
    din("sawT16", (D, 3 * D), F16)
    din("saoT16", (D, D), F16)
    for nm in ["cqT16", "ckT16", "cvT16", "w1T16", "w2T16", "w3T16", "w4T16"]:
        din(nm, (D, D), F16)
    din("coT64_16", (HD, H, D), F16)
    din("sa_in_b16", (3 * D,), F16)
    for nm in ["cv_b16", "co_b16", "b1_16", "b2_16", "b3_16", "b4_16"]:
        din(nm, (D,), F16)
    din("cq_bT", (P, KT), F32)
    din("ck_bT", (P, KT), F32)
    for lnm in ["n1v", "n2v", "n3v", "n1t", "n2t", "n3t"]:
        din(lnm + "_g16", (D,), F16)
        din(lnm + "_b16", (D,), F16)
    for s in ["img", "text"]:
        din(f"xT16_{s}", (D, R), F16)
        din(f"rowsb_{s}", (R, D), F32)
    io["img_out"] = nc.dram_tensor("img_out", [B, P, D], F32,
                                   kind="ExternalOutput").ap()
    io["text_out"] = nc.dram_tensor("text_out", [B, P, D], F32,
                                    kind="ExternalOutput").ap()

    with tile.TileContext(nc, num_cores=NC) as tc:
        emit_full(tc, io, NC, triv)
    nc.finalize()
    return nc


LAST_RESULT = None


def kernel(**inputs):
    NC = 8
    inputs = {k: np.asarray(v) for k, v in inputs.items()}
    L = inputs["img_input"].shape[1]
    TC = L // NC
    triv = _triviality(inputs)
    nc = make_nc(NC, triv)
    common, per_core = _host_prep(inputs, NC)
    in_maps = [dict(common, **pc) for pc in per_core]
    res = bass_utils.run_bass_kernel_spmd(nc, in_maps, core_ids=list(range(NC)))
    global LAST_RESULT
    LAST_RESULT = res
    img = np.empty((B, L, D), np.float32)
    text = np.empty((B, L, D), np.float32)
    for ci in range(NC):
        img[:, ci * TC:(ci + 1) * TC, :] = res.results[ci]["img_out"]
        text[:, ci * TC:(ci + 1) * TC, :] = res.results[ci]["text_out"]
    return img, text

